# revision 37
# baseline (speedup 1.0000x reference)
"""BitSwiGLU Trainium2 kernel (8 NeuronCores).

Math (per bit_linear, forward values):
    gamma_x = clip(max|x_row|, 1e-5);  k = rne(x * 127/gamma_x)  in [-127,127]
    gamma_w = clip(mean|w|, 1e-5);    t = sign(w) * (|w| > 0.5*gamma_w)  in {-1,0,1}
    y = (k @ t.T) * (gamma_x*gamma_w/127) + b

k is exactly representable in bf16 and 2*t in fp8e4; the TensorEngine
accumulates (bf16 x fp8) products in fp32 PSUM, so k @ (2t).T is EXACT
integer math at full PE speed. All scales are applied per-token
(per-partition) at PSUM eviction; the ternary 2x folds into them (/254).

Sharding: data-parallel matmuls (8192 tokens split 1024/core, each core
computes its tokens against the full weights), but tensor-parallel weight
PREP: core i reads only rows [1024*i, 1024*(i+1)) of gate_w/val_w and
columns of out_w (fp32), computes |w|-sum partials (AllReduce -> exact
global gamma = mean|w|), ternarizes its shard, transposes it into matmul
layout ([d, h] for gate/val, [h, d] for out_w) via the DMA XBAR in bf16,
casts to fp8e4 and AllGathers the compact ternary shards. This removes
the 8x-redundant fp32 weight reads (2 x 192 MiB/core in the pure-DP
version) that made weight prep DMA-bound. The gate/val gathers are
split in h-halves so mm1 can start before the full gather lands.

silu(y) is computed as y * sigmoid(y) (Sigmoid on ScalarE). Biases are
zero in this problem; gate/val biases are asserted zero host-side and
out_b is added on host.
"""

import numpy as np

import concourse.bass as bass
import concourse.mybir as mybir
import concourse.tile as tile
from concourse import bacc
from concourse import bass_isa
from concourse.bass_utils import run_bass_kernel_spmd

F32 = mybir.dt.float32
BF16 = mybir.dt.bfloat16
FP8 = mybir.dt.float8e4
AF = mybir.ActivationFunctionType
OP = mybir.AluOpType
AX = mybir.AxisListType

MAGIC = 12582912.0  # 1.5 * 2**23 : (v + MAGIC) - MAGIC == rne(v) for |v| < 2**22

N_CORES = 8
RG = [[i for i in range(N_CORES)]]  # replica group: all cores


def _build(T, D, H, n_cores=N_CORES):
    """Build + compile the per-core Bass program (SPMD: all cores run the
    same program; per-core inputs differ: x token-shard + weight shards)."""
    SH = H // n_cores
    nc = bacc.Bacc("TRN2", target_bir_lowering=False, debug=False,
                   num_devices=n_cores)
    x_d = nc.dram_tensor("x", [T, D], F32, kind="ExternalInput")
    gw_d = nc.dram_tensor("gw", [SH, D], F32, kind="ExternalInput")
    vw_d = nc.dram_tensor("vw", [SH, D], F32, kind="ExternalInput")
    ow_d = nc.dram_tensor("ow", [D, SH], F32, kind="ExternalInput")
    out_d = nc.dram_tensor("out", [T, D], F32, kind="ExternalOutput")

    with tile.TileContext(nc) as tc:
        _body(tc, x_d, gw_d, vw_d, ow_d, out_d, T=T, D=D, H=H, SH=SH)
    nc.compile()
    return nc


def _body(tc, x_d, gw_d, vw_d, ow_d, out_d, *, T, D, H, SH):
    nc = tc.nc
    KD = D // 128      # contraction chunks, mm1 (16)
    KH = H // 128      # contraction chunks, mm2 (64)
    ND = D // 512      # d_out 512-chunks (mm2 output tiles) (4)
    MT = T // 128      # token chunks (8)
    RSH = SH // 128    # gate/val shard row-chunks (8)
    RD = D // 128      # out_w shard row-chunks (16)
    QD = D // 128      # d-chunks of transposed gate/val layout (16)
    HB = SH // 128     # h-blocks of transposed out_w layout (8)
    NS = N_CORES       # weight shards
    MHALF = 2          # token chunks per PSUM wave
    CQ = 512           # h-quant processing chunk
    NQ = H // CQ
    WCNT = float(H * D)  # element count of each weight matrix (mean divisor)

    Xv = x_d.ap().rearrange("(m p) d -> m p d", p=128)
    Ov = out_d.ap().rearrange("(m p) d -> m p d", p=128)
    Gv = gw_d.ap().rearrange("(r p) c -> r p c", p=128)
    Vv = vw_d.ap().rearrange("(r p) c -> r p c", p=128)
    Owv = ow_d.ap().rearrange("(r p) c -> r p c", p=128)

    with (
        tc.tile_pool(name="persist", bufs=1) as pp,
        tc.tile_pool(name="psp", bufs=8, space="PSUM") as psp,
        tc.tile_pool(name="drp", bufs=1, space="DRAM") as drp,
    ):
        # ---- DRAM scratch ----
        # own ternary shards (transposed to matmul layout, fp8 {-2,0,2});
        # gate+val share one tensor per h-half so ONE AllGather moves both
        # (the ~100us fixed ring overhead per collective dominates).
        gvsh = [drp.tile([2, D, SH // 2], FP8, tag=f"gvsh{a}",
                         name=f"gvsh{a}") for a in range(2)]
        osh = drp.tile([2, SH, D // 2], FP8, tag="osh", name="osh")
        # all-gathered ternary weights
        gvall = [drp.tile([NS, 2, D, SH // 2], FP8, tag=f"gvall{a}",
                          name=f"gvall{a}", addr_space="Shared")
                 for a in range(2)]
        oall = drp.tile([NS, 2, SH, D // 2], FP8, tag="oall", name="oall",
                        addr_space="Shared")
        # gamma partial-sum bounce buffers (gate+val in one early
        # AllReduce; out_w in a second, off the mm1 critical path)
        gvsum_in = drp.tile([128, 2], F32, tag="gvsum_in", name="gvsum_in")
        gvsum_out = drp.tile([128, 2], F32, tag="gvsum_out",
                             name="gvsum_out", addr_space="Shared")
        osum_in = drp.tile([128, 1], F32, tag="osum_in", name="osum_in")
        osum_out = drp.tile([128, 1], F32, tag="osum_out", name="osum_out",
                            addr_space="Shared")
        # h spill
        h_d = drp.tile([MT, 128, H], F32, tag="h", name="h_d")

        # ---- persistent SBUF ----
        s1, s12, gx_l, hmax, s_out = [], [], [], [], []
        for m in range(MT):
            for nm, lst in (("s1", s1), ("s12", s12), ("gx", gx_l),
                            ("hmax", hmax), ("so", s_out)):
                t = pp.tile([128, 1], F32, tag=f"{nm}{m}", name=f"{nm}{m}")
                lst.append(t)
        hp = [pp.tile([128, H // 512], F32, tag=f"hp{m}", name=f"hp{m}")
              for m in range(MT)]
        # per-partition |w| partial sums: cols 0..RSH-1 gate, RSH..2RSH-1
        # val, 2RSH..2RSH+RD-1 out
        parts = pp.tile([128, 2 * RSH + RD], F32, tag="parts", name="parts")
        sums_gv = pp.tile([128, 2], F32, tag="sums_gv", name="sums_gv")
        sums_o = pp.tile([128, 1], F32, tag="sums_o", name="sums_o")
        gam, thr, nthr = {}, {}, {}
        for j, nm in enumerate(("g", "v", "o")):
            gam[nm] = pp.tile([128, 1], F32, tag=f"gam_{nm}", name=f"gam_{nm}")
            thr[nm] = pp.tile([128, 1], F32, tag=f"thr_{nm}", name=f"thr_{nm}")
            nthr[nm] = pp.tile([128, 1], F32, tag=f"nthr_{nm}",
                               name=f"nthr_{nm}")

        with tc.tile_pool(name="kxp", bufs=1) as kxp:
            # kxT[p=d, k, t] = k_x[t, k*128+p]
            kxT = kxp.tile([128, KD, T], BF16, tag="kxT")

            with tc.tile_pool(name="wp", bufs=3) as wp:
                # ---- |w| partial sums of the own shard ----
                def gamma_cols(Wv, R, C, col0):
                    for r in range(R):
                        wt = wp.tile([128, 2048], F32, tag="w_in")
                        nc.sync.dma_start(out=wt[:, :C], in_=Wv[r])
                        nc.vector.tensor_reduce(
                            out=parts[:, col0 + r:col0 + r + 1],
                            in_=wt[:, :C], axis=AX.X, op=OP.add,
                            apply_absolute_value=True)

                def thr_block(nm, src_ap):
                    nc.vector.tensor_scalar(out=gam[nm][:, :], in0=src_ap,
                                            scalar1=1.0 / WCNT,
                                            scalar2=1e-5, op0=OP.mult,
                                            op1=OP.max)
                    nc.vector.tensor_scalar_mul(out=thr[nm][:, :],
                                                in0=gam[nm][:, :],
                                                scalar1=0.5)
                    nc.vector.tensor_scalar_mul(out=nthr[nm][:, :],
                                                in0=thr[nm][:, :],
                                                scalar1=-1.0)

                # gate+val gamma partials -> one small AllReduce; all the
                # scalar-sized DMAs ride the idle gpsimd queue so they
                # never head-of-line block the SP weight stream
                gamma_cols(Gv, RSH, D, 0)
                gamma_cols(Vv, RSH, D, RSH)
                psum_gv = pp.tile([128, 2], F32, tag="psum_gv",
                                  name="psum_gv")
                nc.vector.tensor_reduce(out=psum_gv[:, 0:1],
                                        in_=parts[:, 0:RSH], axis=AX.X,
                                        op=OP.add)
                nc.vector.tensor_reduce(out=psum_gv[:, 1:2],
                                        in_=parts[:, RSH:2 * RSH],
                                        axis=AX.X, op=OP.add)
                nc.gpsimd.dma_start(out=gvsum_in[:, :], in_=psum_gv[:, :])
                nc.gpsimd.collective_compute(
                    "AllReduce", OP.add, replica_groups=RG,
                    ins=[gvsum_in.opt()], outs=[gvsum_out.opt()])
                nc.gpsimd.dma_start(out=sums_gv[:, :], in_=gvsum_out[:, :])
                nc.gpsimd.partition_all_reduce(sums_gv[:, :],
                                               sums_gv[:, :], 128,
                                               bass_isa.ReduceOp.add)
                thr_block("g", sums_gv[:, 0:1])
                thr_block("v", sums_gv[:, 1:2])

                # ternarize own shard rows -> bf16, XBAR-transpose into a
                # per-half staging tile, cast to fp8, write to DRAM shard
                def tern_rows(Wv, r0, r1, C, wT, th, nth, dve):
                    for r in range(r0, r1):
                        wt = wp.tile([128, 2048], F32, tag="w_in")
                        nc.sync.dma_start(out=wt[:, :C], in_=Wv[r])
                        tq = wp.tile([128, 2048], BF16, tag="q_tq")
                        if dve:
                            mp = wp.tile([128, 2048], BF16, tag="q_mm",
                                         bufs=2)
                            nc.vector.tensor_scalar(
                                out=mp[:, :C], in0=wt[:, :C],
                                scalar1=th[:, :], scalar2=2.0,
                                op0=OP.is_gt, op1=OP.mult)
                            mn = wp.tile([128, 2048], BF16, tag="q_mm",
                                         bufs=2)
                            nc.vector.tensor_scalar(
                                out=mn[:, :C], in0=wt[:, :C],
                                scalar1=nth[:, :], scalar2=2.0,
                                op0=OP.is_lt, op1=OP.mult)
                            nc.vector.tensor_sub(out=tq[:, :C],
                                                 in0=mp[:, :C],
                                                 in1=mn[:, :C])
                        else:
                            sp = wp.tile([128, 2048], BF16, tag="q_sp",
                                         bufs=2)
                            nc.scalar.activation(out=sp[:, :C],
                                                 in_=wt[:, :C],
                                                 func=AF.Sign,
                                                 bias=nth[:, :])
                            sn = wp.tile([128, 2048], BF16, tag="q_sn",
                                         bufs=2)
                            nc.scalar.activation(out=sn[:, :C],
                                                 in_=wt[:, :C],
                                                 func=AF.Sign,
                                                 bias=th[:, :])
                            nc.vector.tensor_add(out=tq[:, :C],
                                                 in0=sp[:, :C],
                                                 in1=sn[:, :C])
                        # multi-tile XBAR transpose: out[p, q, c] =
                        # tq[c, q*128+p]
                        cs = (r - r0) * 128
                        nc.scalar.dma_start(out=wT[:, :, cs:cs + 128],
                                            in_=tq[:, :C], transpose=True)

                def tern_unit(Wv, r0, r1, C, nq, th, nth, dve, dst_view):
                    ncols = (r1 - r0) * 128
                    wT = wp.tile([128, nq * ncols], BF16, tag="wT", bufs=2)
                    wTv = wT.rearrange("p (q c) -> p q c", c=ncols)
                    tern_rows(Wv, r0, r1, C, wTv, th, nth, dve)
                    f8 = wp.tile([128, nq * ncols], FP8, tag="f8w", bufs=2)
                    f8v = f8.rearrange("p (q c) -> p q c", c=ncols)
                    nc.scalar.activation(out=f8v[:, :, :], in_=wTv[:, :, :],
                                         func=AF.Copy)
                    nc.scalar.dma_start(out=dst_view, in_=f8v[:, :, :])

                R2 = RSH // 2
                gvshv = [gvsh[a].rearrange("t (q p) c -> t p q c", p=128)
                         for a in range(2)]
                oshv = osh.rearrange("t (q p) c -> t p q c", p=128)

                # first gate/val halves -> ONE combined AllGather (the mm1
                # critical path)
                tern_unit(Gv, 0, R2, D, QD, thr["g"], nthr["g"], False,
                          gvshv[0][0])
                tern_unit(Vv, 0, R2, D, QD, thr["v"], nthr["v"], False,
                          gvshv[0][1])
                nc.gpsimd.collective_compute(
                    "AllGather", OP.bypass, replica_groups=RG,
                    ins=[gvsh[0].opt()], outs=[gvall[0].opt()])

                # out_w gamma + its own AllReduce (cc-pipe slot after the
                # first gather)
                gamma_cols(Owv, RD, SH, 2 * RSH)
                nc.vector.tensor_reduce(out=sums_o[:, :],
                                        in_=parts[:, 2 * RSH:2 * RSH + RD],
                                        axis=AX.X, op=OP.add)
                nc.gpsimd.dma_start(out=osum_in[:, :], in_=sums_o[:, :])
                nc.gpsimd.collective_compute(
                    "AllReduce", OP.add, replica_groups=RG,
                    ins=[osum_in.opt()], outs=[osum_out.opt()])
                nc.gpsimd.dma_start(out=sums_o[:, :], in_=osum_out[:, :])
                nc.gpsimd.partition_all_reduce(sums_o[:, :], sums_o[:, :],
                                               128, bass_isa.ReduceOp.add)
                thr_block("o", sums_o[:, :])

                # ---- x quantization (loads on SP after the a0 weight
                # stream; scale-copy + rne on DVE so the Act queue stays
                # free for the Sign/transpose/flush chain) ----
                with tc.tile_pool(name="xst", bufs=3) as xst:
                    for m in range(MT):
                        xt = xst.tile([128, D], F32, tag="x_in")
                        nc.sync.dma_start(out=xt[:, :], in_=Xv[m])
                        gx = gx_l[m]
                        nc.vector.tensor_reduce(out=gx[:, :], in_=xt[:, :],
                                                axis=AX.X, op=OP.max,
                                                apply_absolute_value=True)
                        nc.vector.tensor_scalar_max(out=gx[:, :],
                                                    in0=gx[:, :],
                                                    scalar1=1e-5)
                        rcp = xst.tile([128, 1], F32, tag="rcpx")
                        nc.vector.reciprocal(out=rcp[:, :], in_=gx[:, :])
                        sx = xst.tile([128, 1], F32, tag="sx")
                        nc.vector.tensor_scalar_mul(out=sx[:, :],
                                                    in0=rcp[:, :],
                                                    scalar1=127.0)
                        # k_x = rne(x * sx) -> bf16 (exact small ints)
                        xs = xst.tile([128, D], F32, tag="x_sc")
                        nc.vector.tensor_scalar_mul(out=xs[:, :],
                                                    in0=xt[:, :],
                                                    scalar1=sx[:, :])
                        kx = xst.tile([128, D], BF16, tag="kx")
                        nc.vector.tensor_scalar(out=kx[:, :], in0=xs[:, :],
                                                scalar1=MAGIC, scalar2=MAGIC,
                                                op0=OP.add, op1=OP.subtract)
                        nc.sync.dma_start(out=kxT[:, :,
                                                  m * 128:(m + 1) * 128],
                                          in_=kx[:, :], transpose=True)

                # per-token eviction scales; /254 folds the ternary 2x
                for m in range(MT):
                    nc.vector.tensor_scalar(out=s1[m][:, :],
                                            in0=gx_l[m][:, :],
                                            scalar1=gam["g"][:, :],
                                            scalar2=1.0 / 254.0,
                                            op0=OP.mult, op1=OP.mult)
                    s2 = wp.tile([128, 1], F32, tag="s2tmp")
                    nc.vector.tensor_scalar(out=s2[:, :], in0=gx_l[m][:, :],
                                            scalar1=gam["v"][:, :],
                                            scalar2=1.0 / 254.0,
                                            op0=OP.mult, op1=OP.mult)
                    nc.vector.tensor_mul(out=s12[m][:, :], in0=s1[m][:, :],
                                         in1=s2[:, :])

                # second gate/val halves -> combined AllGather
                tern_unit(Gv, R2, RSH, D, QD, thr["g"], nthr["g"], False,
                          gvshv[1][0])
                tern_unit(Vv, R2, RSH, D, QD, thr["v"], nthr["v"], False,
                          gvshv[1][1])
                nc.gpsimd.collective_compute(
                    "AllGather", OP.bypass, replica_groups=RG,
                    ins=[gvsh[1].opt()], outs=[gvall[1].opt()])

                # out_w shard: [D, SH] fp32 -> ternary [SH, D] (h-major),
                # d-halves combined into one AllGather
                for a in range(2):
                    tern_unit(Owv, a * (RD // 2), (a + 1) * (RD // 2), SH,
                              HB, thr["o"], nthr["o"], False, oshv[a])
                nc.gpsimd.collective_compute(
                    "AllGather", OP.bypass, replica_groups=RG,
                    ins=[osh.opt()], outs=[oall.opt()])

            # ---------------- mm1: gate/val matmuls + h ----------------
            # panel (s, a) covers h rows [s*SH + a*512, ...+512)
            with tc.tile_pool(name="m1p", bufs=2) as m1p:
                gvallv = [gvall[a].rearrange("s t (q p) c -> (s t) p q c",
                                             p=128) for a in range(2)]
                for a in range(2):
                    for s in range(NS):
                        n = 2 * s + a  # h 512-panel index
                        wg_s = m1p.tile([128, KD, 512], FP8, tag="wg_s")
                        wv_s = m1p.tile([128, KD, 512], FP8, tag="wv_s")
                        nc.sync.dma_start(out=wg_s[:, :, :],
                                          in_=gvallv[a][2 * s])
                        nc.sync.dma_start(out=wv_s[:, :, :],
                                          in_=gvallv[a][2 * s + 1])
                        for half in range(MT // MHALF):
                            ms = range(half * MHALF, (half + 1) * MHALF)
                            pg = {m: psp.tile([128, 512], F32, tag="ps",
                                              name=f"pg{n}_{m}")
                                  for m in ms}
                            pv = {m: psp.tile([128, 512], F32, tag="ps",
                                              name=f"pv{n}_{m}")
                                  for m in ms}
                            for k in range(KD):
                                for m in ms:
                                    lhsT = kxT[:, k, m * 128:(m + 1) * 128]
                                    nc.tensor.matmul(pg[m][:, :], lhsT=lhsT,
                                                     rhs=wg_s[:, k, :],
                                                     start=(k == 0),
                                                     stop=(k == KD - 1))
                                    nc.tensor.matmul(pv[m][:, :], lhsT=lhsT,
                                                     rhs=wv_s[:, k, :],
                                                     start=(k == 0),
                                                     stop=(k == KD - 1))
                            for m in ms:
                                A = m1p.tile([128, 512], F32, tag="Asb",
                                             bufs=MHALF + 2, name=f"A{n}_{m}")
                                nc.scalar.activation(out=A[:, :],
                                                     in_=pg[m][:, :],
                                                     func=AF.Sigmoid,
                                                     scale=s1[m][:, :])
                                B = m1p.tile([128, 512], F32, tag="Bsb",
                                             bufs=MHALF + 2, name=f"B{n}_{m}")
                                nc.scalar.activation(out=B[:, :],
                                                     in_=pg[m][:, :],
                                                     func=AF.Copy,
                                                     scale=s12[m][:, :])
                                tmp = m1p.tile([128, 512], F32, tag="tmp",
                                               bufs=4, name=f"tmp{n}_{m}")
                                nc.vector.tensor_mul(out=tmp[:, :],
                                                     in0=pv[m][:, :],
                                                     in1=B[:, :])
                                hs = m1p.tile([128, 512], F32, tag="hsl",
                                              bufs=4, name=f"hs{n}_{m}")
                                nc.vector.tensor_mul(out=hs[:, :],
                                                     in0=A[:, :],
                                                     in1=tmp[:, :])
                                nc.vector.tensor_reduce(
                                    out=hp[m][:, n:n + 1], in_=hs[:, :],
                                    axis=AX.X, op=OP.max,
                                    apply_absolute_value=True)
                                nc.sync.dma_start(
                                    out=h_d[m, :, n * 512:(n + 1) * 512],
                                    in_=hs[:, :])

        # ---------------- h quantization + mm2 (interleaved per token
        # chunk: h-quant(m+1) overlaps mm2(m) on the PE) ----------------
        with tc.tile_pool(name="khp", bufs=1) as khp:
            # mm2 weights resident in SBUF (16 MiB fp8 = 128KiB/partition),
            # loaded in 8 k-chunks so mm2(m=0) can start on chunk 0.
            # oall as [8192 h, 2048 d]: row h = s*SH + hb*128 + p
            wo_all = khp.tile([128, KH, D], FP8, tag="wo_all")
            # one k-chunk of 8 == one shard s: index per (shard, d-half)
            oallv = oall.rearrange("s t (k p) c -> s t p k c", p=128)
            D2 = D // 2
            for kk in range(8):
                sl = slice(kk * (KH // 8), (kk + 1) * (KH // 8))
                for a in range(2):
                    nc.sync.dma_start(
                        out=wo_all[:, sl, a * D2:(a + 1) * D2],
                        in_=oallv[kk][a])
            with tc.tile_pool(name="hqp", bufs=3) as hqp:
                for m in range(MT):
                    nc.vector.tensor_reduce(out=hmax[m][:, :],
                                            in_=hp[m][:, :], axis=AX.X,
                                            op=OP.max)
                    gh = hqp.tile([128, 1], F32, tag="gh")
                    nc.vector.tensor_scalar_max(out=gh[:, :],
                                                in0=hmax[m][:, :],
                                                scalar1=1e-5)
                    rch = hqp.tile([128, 1], F32, tag="rch")
                    nc.vector.reciprocal(out=rch[:, :], in_=gh[:, :])
                    sh = hqp.tile([128, 1], F32, tag="sh")
                    nc.vector.tensor_scalar_mul(out=sh[:, :], in0=rch[:, :],
                                                scalar1=127.0)
                    nc.vector.tensor_scalar(out=s_out[m][:, :],
                                            in0=gh[:, :],
                                            scalar1=gam["o"][:, :],
                                            scalar2=1.0 / 254.0,
                                            op0=OP.mult, op1=OP.mult)
                    kT = hqp.tile([128, KH, 128], BF16, tag="khT", bufs=2,
                                  name=f"khT{m}")
                    for q in range(NQ):
                        hc = hqp.tile([128, CQ], F32, tag="h_rd")
                        nc.gpsimd.dma_start(
                            out=hc[:, :],
                            in_=h_d[m, :, q * CQ:(q + 1) * CQ])
                        hsc = hqp.tile([128, CQ], F32, tag="h_sc")
                        nc.scalar.activation(out=hsc[:, :], in_=hc[:, :],
                                             func=AF.Copy, scale=sh[:, :])
                        kh = hqp.tile([128, CQ], BF16, tag="kh")
                        nc.vector.tensor_scalar(out=kh[:, :], in0=hsc[:, :],
                                                scalar1=MAGIC, scalar2=MAGIC,
                                                op0=OP.add, op1=OP.subtract)
                        nc.sync.dma_start(
                            out=kT[:, q * (CQ // 128):(q + 1) * (CQ // 128),
                                   :],
                            in_=kh[:, :], transpose=True)
                    po = [psp.tile([128, 512], F32, tag="ps",
                                   name=f"po{m}_{c}") for c in range(ND)]
                    for k in range(KH):
                        for c in range(ND):
                            nc.tensor.matmul(
                                po[c][:, :], lhsT=kT[:, k, :],
                                rhs=wo_all[:, k, c * 512:(c + 1) * 512],
                                start=(k == 0), stop=(k == KH - 1))
                    for c in range(ND):
                        ot = hqp.tile([128, 512], F32, tag="ot", bufs=4,
                                      name=f"ot{m}_{c}")
                        nc.vector.tensor_scalar_mul(out=ot[:, :],
                                                    in0=po[c][:, :],
                                                    scalar1=s_out[m][:, :])
                        nc.sync.dma_start(
                            out=Ov[m][:, c * 512:(c + 1) * 512],
                            in_=ot[:, :])


_NC_CACHE = {}


def _get_nc(T, D, H):
    key = (T, D, H)
    if key not in _NC_CACHE:
        _NC_CACHE[key] = _build(T, D, H)
    return _NC_CACHE[key]


def kernel(x, gate_w, gate_b, val_w, val_b, out_w, out_b, _trace=False):
    x = np.ascontiguousarray(np.asarray(x), dtype=np.float32)
    gate_w = np.ascontiguousarray(np.asarray(gate_w), dtype=np.float32)
    val_w = np.ascontiguousarray(np.asarray(val_w), dtype=np.float32)
    out_w = np.ascontiguousarray(np.asarray(out_w), dtype=np.float32)
    gate_b = np.asarray(gate_b)
    val_b = np.asarray(val_b)
    out_b = np.asarray(out_b)
    assert not np.any(gate_b) and not np.any(val_b), (
        "device kernel folds silu(y+b) with b=0; nonzero gate/val bias "
        "not supported")

    orig_shape = x.shape
    xf = x.reshape(-1, x.shape[-1])
    n_tok, d = xf.shape
    h = gate_w.shape[0]
    t_core = n_tok // N_CORES
    sh = h // N_CORES

    nc = _get_nc(t_core, d, h)
    in_maps = [
        {
            "x": xf[i * t_core:(i + 1) * t_core],
            "gw": gate_w[i * sh:(i + 1) * sh],
            "vw": val_w[i * sh:(i + 1) * sh],
            "ow": np.ascontiguousarray(out_w[:, i * sh:(i + 1) * sh]),
        }
        for i in range(N_CORES)
    ]
    res = run_bass_kernel_spmd(nc, in_maps, core_ids=list(range(N_CORES)),
                               trace=_trace)
    out = np.concatenate([res.results[i]["out"] for i in range(N_CORES)],
                         axis=0)
    out = out + out_b[None, :].astype(np.float32)
    kernel._last_results = res
    return out.reshape(orig_shape)


# revision 38
# speedup vs baseline: 1.2185x; 1.2185x over previous
"""BitSwiGLU Trainium2 kernel (8 NeuronCores).

Math (per bit_linear, forward values):
    gamma_x = clip(max|x_row|, 1e-5);  k = rne(x * 127/gamma_x)  in [-127,127]
    gamma_w = clip(mean|w|, 1e-5);    t = sign(w) * (|w| > 0.5*gamma_w)  in {-1,0,1}
    y = (k @ t.T) * (gamma_x*gamma_w/127) + b

k is exactly representable in bf16 and 2*t in fp8e4; the TensorEngine
accumulates (bf16 x fp8) products in fp32 PSUM, so k @ (2t).T is EXACT
integer math at full PE speed. All scales are applied per-token
(per-partition) at PSUM eviction; the ternary 2x folds into them (/254).

Sharding: data-parallel matmuls (8192 tokens split 1024/core, each core
computes its tokens against the full weights), but tensor-parallel weight
PREP: core i reads only rows [1024*i, 1024*(i+1)) of gate_w/val_w and
columns of out_w (fp32), computes |w|-sum partials (AllReduce -> exact
global gamma = mean|w|), ternarizes its shard, transposes it into matmul
layout ([d, h] for gate/val, [h, d] for out_w) via the DMA XBAR in bf16,
casts to fp8e4 and AllGathers the compact ternary shards. This removes
the 8x-redundant fp32 weight reads (2 x 192 MiB/core in the pure-DP
version) that made weight prep DMA-bound. The gate/val gathers are
split in h-halves so mm1 can start before the full gather lands.

silu(y) is computed as y * sigmoid(y) (Sigmoid on ScalarE). Biases are
zero in this problem; gate/val biases are asserted zero host-side and
out_b is added on host.
"""

import numpy as np

import concourse.bass as bass
import concourse.mybir as mybir
import concourse.tile as tile
from concourse import bacc
from concourse import bass_isa
from concourse.bass_utils import run_bass_kernel_spmd

F32 = mybir.dt.float32
BF16 = mybir.dt.bfloat16
FP8 = mybir.dt.float8e4
AF = mybir.ActivationFunctionType
OP = mybir.AluOpType
AX = mybir.AxisListType

MAGIC = 12582912.0  # 1.5 * 2**23 : (v + MAGIC) - MAGIC == rne(v) for |v| < 2**22

N_CORES = 8
RG = [[i for i in range(N_CORES)]]  # replica group: all cores


def _build(T, D, H, n_cores=N_CORES):
    """Build + compile the per-core Bass program (SPMD: all cores run the
    same program; per-core inputs differ: x token-shard + weight shards)."""
    SH = H // n_cores
    nc = bacc.Bacc("TRN2", target_bir_lowering=False, debug=False,
                   num_devices=n_cores)
    x_d = nc.dram_tensor("x", [T, D], F32, kind="ExternalInput")
    gw_d = nc.dram_tensor("gw", [SH, D], F32, kind="ExternalInput")
    vw_d = nc.dram_tensor("vw", [SH, D], F32, kind="ExternalInput")
    ow_d = nc.dram_tensor("ow", [D, SH], F32, kind="ExternalInput")
    out_d = nc.dram_tensor("out", [T, D], F32, kind="ExternalOutput")

    with tile.TileContext(nc) as tc:
        _body(tc, x_d, gw_d, vw_d, ow_d, out_d, T=T, D=D, H=H, SH=SH)
    nc.compile()
    return nc


def _body(tc, x_d, gw_d, vw_d, ow_d, out_d, *, T, D, H, SH):
    nc = tc.nc
    KD = D // 128      # contraction chunks, mm1 (16)
    KH = H // 128      # contraction chunks, mm2 (64)
    ND = D // 512      # d_out 512-chunks (mm2 output tiles) (4)
    MT = T // 128      # token chunks (8)
    RSH = SH // 128    # gate/val shard row-chunks (8)
    RD = D // 128      # out_w shard row-chunks (16)
    QD = D // 128      # d-chunks of transposed gate/val layout (16)
    HB = SH // 128     # h-blocks of transposed out_w layout (8)
    NS = N_CORES       # weight shards
    MHALF = 2          # token chunks per PSUM wave
    CQ = 512           # h-quant processing chunk
    NQ = H // CQ
    WCNT = float(H * D)  # element count of each weight matrix (mean divisor)

    Xv = x_d.ap().rearrange("(m p) d -> m p d", p=128)
    Ov = out_d.ap().rearrange("(m p) d -> m p d", p=128)
    Gv = gw_d.ap().rearrange("(r p) c -> r p c", p=128)
    Vv = vw_d.ap().rearrange("(r p) c -> r p c", p=128)
    Owv = ow_d.ap().rearrange("(r p) c -> r p c", p=128)

    with (
        tc.tile_pool(name="persist", bufs=1) as pp,
        tc.tile_pool(name="psp", bufs=8, space="PSUM") as psp,
        tc.tile_pool(name="drp", bufs=1, space="DRAM") as drp,
    ):
        # ---- DRAM scratch ----
        # own ternary shards (transposed to matmul layout, fp8 {-2,0,2});
        # gate+val share one tensor per h-half so ONE AllGather moves both
        # (the ~100us fixed ring overhead per collective dominates).
        gvsh = [drp.tile([2, D, SH // 2], FP8, tag=f"gvsh{a}",
                         name=f"gvsh{a}") for a in range(2)]
        osh = drp.tile([2, SH, D // 2], FP8, tag="osh", name="osh")
        # all-gathered ternary weights
        gvall = [drp.tile([NS, 2, D, SH // 2], FP8, tag=f"gvall{a}",
                          name=f"gvall{a}", addr_space="Shared")
                 for a in range(2)]
        oall = drp.tile([NS, 2, SH, D // 2], FP8, tag="oall", name="oall",
                        addr_space="Shared")
        # gamma partial-sum bounce buffers (gate+val in one early
        # AllReduce; out_w in a second, off the mm1 critical path)
        gvsum_in = drp.tile([128, 2], F32, tag="gvsum_in", name="gvsum_in")
        gvsum_out = drp.tile([128, 2], F32, tag="gvsum_out",
                             name="gvsum_out", addr_space="Shared")
        osum_in = drp.tile([128, 1], F32, tag="osum_in", name="osum_in")
        osum_out = drp.tile([128, 1], F32, tag="osum_out", name="osum_out",
                            addr_space="Shared")
        # h spill
        h_d = drp.tile([MT, 128, H], F32, tag="h", name="h_d")

        # ---- persistent SBUF ----
        s1, s12, gx_l, hmax, s_out = [], [], [], [], []
        for m in range(MT):
            for nm, lst in (("s1", s1), ("s12", s12), ("gx", gx_l),
                            ("hmax", hmax), ("so", s_out)):
                t = pp.tile([128, 1], F32, tag=f"{nm}{m}", name=f"{nm}{m}")
                lst.append(t)
        hp = [pp.tile([128, H // 512], F32, tag=f"hp{m}", name=f"hp{m}")
              for m in range(MT)]
        # per-partition |w| partial sums: cols 0..RSH-1 gate, RSH..2RSH-1
        # val, 2RSH..2RSH+RD-1 out
        parts = pp.tile([128, 2 * RSH + RD], F32, tag="parts", name="parts")
        sums_gv = pp.tile([128, 2], F32, tag="sums_gv", name="sums_gv")
        sums_o = pp.tile([128, 1], F32, tag="sums_o", name="sums_o")
        gam, thr, nthr = {}, {}, {}
        for j, nm in enumerate(("g", "v", "o")):
            gam[nm] = pp.tile([128, 1], F32, tag=f"gam_{nm}", name=f"gam_{nm}")
            thr[nm] = pp.tile([128, 1], F32, tag=f"thr_{nm}", name=f"thr_{nm}")
            nthr[nm] = pp.tile([128, 1], F32, tag=f"nthr_{nm}",
                               name=f"nthr_{nm}")

        with tc.tile_pool(name="kxp", bufs=1,
                          side="right") as kxp:
            # kxT[p=d, k, t] = k_x[t, k*128+p]
            kxT = kxp.tile([128, KD, T], BF16, tag="kxT")

            with tc.tile_pool(name="wp", bufs=3) as wp:
                # ---- |w| partial sums of the own shard ----
                def gamma_cols(Wv, R, C, col0):
                    for r in range(R):
                        wt = wp.tile([128, 2048], F32, tag="w_in")
                        nc.sync.dma_start(out=wt[:, :C], in_=Wv[r])
                        nc.vector.tensor_reduce(
                            out=parts[:, col0 + r:col0 + r + 1],
                            in_=wt[:, :C], axis=AX.X, op=OP.add,
                            apply_absolute_value=True)

                def thr_block(nm, src_ap):
                    nc.vector.tensor_scalar(out=gam[nm][:, :], in0=src_ap,
                                            scalar1=1.0 / WCNT,
                                            scalar2=1e-5, op0=OP.mult,
                                            op1=OP.max)
                    nc.vector.tensor_scalar_mul(out=thr[nm][:, :],
                                                in0=gam[nm][:, :],
                                                scalar1=0.5)
                    nc.vector.tensor_scalar_mul(out=nthr[nm][:, :],
                                                in0=thr[nm][:, :],
                                                scalar1=-1.0)

                # gate+val gamma partials -> one small AllReduce; all the
                # scalar-sized DMAs ride the idle gpsimd queue so they
                # never head-of-line block the SP weight stream
                gamma_cols(Gv, RSH, D, 0)
                gamma_cols(Vv, RSH, D, RSH)
                psum_gv = pp.tile([128, 2], F32, tag="psum_gv",
                                  name="psum_gv")
                nc.vector.tensor_reduce(out=psum_gv[:, 0:1],
                                        in_=parts[:, 0:RSH], axis=AX.X,
                                        op=OP.add)
                nc.vector.tensor_reduce(out=psum_gv[:, 1:2],
                                        in_=parts[:, RSH:2 * RSH],
                                        axis=AX.X, op=OP.add)
                nc.gpsimd.dma_start(out=gvsum_in[:, :], in_=psum_gv[:, :])
                nc.gpsimd.collective_compute(
                    "AllReduce", OP.add, replica_groups=RG,
                    ins=[gvsum_in.opt()], outs=[gvsum_out.opt()])
                nc.gpsimd.dma_start(out=sums_gv[:, :], in_=gvsum_out[:, :])
                nc.gpsimd.partition_all_reduce(sums_gv[:, :],
                                               sums_gv[:, :], 128,
                                               bass_isa.ReduceOp.add)
                thr_block("g", sums_gv[:, 0:1])
                thr_block("v", sums_gv[:, 1:2])

                # ternarize own shard rows -> bf16, XBAR-transpose into a
                # per-half staging tile, cast to fp8, write to DRAM shard
                def tern_rows(Wv, r0, r1, C, wT, th, nth, dve):
                    for r in range(r0, r1):
                        wt = wp.tile([128, 2048], F32, tag="w_in")
                        nc.sync.dma_start(out=wt[:, :C], in_=Wv[r])
                        tq = wp.tile([128, 2048], BF16, tag="q_tq")
                        if dve:
                            mp = wp.tile([128, 2048], BF16, tag="q_mm",
                                         bufs=2)
                            nc.vector.tensor_scalar(
                                out=mp[:, :C], in0=wt[:, :C],
                                scalar1=th[:, :], scalar2=2.0,
                                op0=OP.is_gt, op1=OP.mult)
                            mn = wp.tile([128, 2048], BF16, tag="q_mm",
                                         bufs=2)
                            nc.vector.tensor_scalar(
                                out=mn[:, :C], in0=wt[:, :C],
                                scalar1=nth[:, :], scalar2=2.0,
                                op0=OP.is_lt, op1=OP.mult)
                            nc.vector.tensor_sub(out=tq[:, :C],
                                                 in0=mp[:, :C],
                                                 in1=mn[:, :C])
                        else:
                            sp = wp.tile([128, 2048], BF16, tag="q_sp",
                                         bufs=2)
                            nc.scalar.activation(out=sp[:, :C],
                                                 in_=wt[:, :C],
                                                 func=AF.Sign,
                                                 bias=nth[:, :])
                            sn = wp.tile([128, 2048], BF16, tag="q_sn",
                                         bufs=2)
                            nc.scalar.activation(out=sn[:, :C],
                                                 in_=wt[:, :C],
                                                 func=AF.Sign,
                                                 bias=th[:, :])
                            nc.vector.tensor_add(out=tq[:, :C],
                                                 in0=sp[:, :C],
                                                 in1=sn[:, :C])
                        # multi-tile XBAR transpose: out[p, q, c] =
                        # tq[c, q*128+p]
                        cs = (r - r0) * 128
                        nc.scalar.dma_start(out=wT[:, :, cs:cs + 128],
                                            in_=tq[:, :C], transpose=True)

                def tern_unit(Wv, r0, r1, C, nq, th, nth, dve, dst_view):
                    ncols = (r1 - r0) * 128
                    wT = wp.tile([128, nq * ncols], BF16, tag="wT", bufs=2)
                    wTv = wT.rearrange("p (q c) -> p q c", c=ncols)
                    tern_rows(Wv, r0, r1, C, wTv, th, nth, dve)
                    f8 = wp.tile([128, nq * ncols], FP8, tag="f8w", bufs=2)
                    f8v = f8.rearrange("p (q c) -> p q c", c=ncols)
                    nc.scalar.activation(out=f8v[:, :, :], in_=wTv[:, :, :],
                                         func=AF.Copy)
                    nc.scalar.dma_start(out=dst_view, in_=f8v[:, :, :])

                R2 = RSH // 2
                gvshv = [gvsh[a].rearrange("t (q p) c -> t p q c", p=128)
                         for a in range(2)]
                oshv = osh.rearrange("t (q p) c -> t p q c", p=128)

                # first gate/val halves -> ONE combined AllGather (the mm1
                # critical path)
                tern_unit(Gv, 0, R2, D, QD, thr["g"], nthr["g"], False,
                          gvshv[0][0])
                tern_unit(Vv, 0, R2, D, QD, thr["v"], nthr["v"], False,
                          gvshv[0][1])
                nc.gpsimd.collective_compute(
                    "AllGather", OP.bypass, replica_groups=RG,
                    ins=[gvsh[0].opt()], outs=[gvall[0].opt()])

                # out_w gamma + its own AllReduce (cc-pipe slot after the
                # first gather)
                gamma_cols(Owv, RD, SH, 2 * RSH)
                nc.vector.tensor_reduce(out=sums_o[:, :],
                                        in_=parts[:, 2 * RSH:2 * RSH + RD],
                                        axis=AX.X, op=OP.add)
                nc.gpsimd.dma_start(out=osum_in[:, :], in_=sums_o[:, :])
                nc.gpsimd.collective_compute(
                    "AllReduce", OP.add, replica_groups=RG,
                    ins=[osum_in.opt()], outs=[osum_out.opt()])
                nc.gpsimd.dma_start(out=sums_o[:, :], in_=osum_out[:, :])
                nc.gpsimd.partition_all_reduce(sums_o[:, :], sums_o[:, :],
                                               128, bass_isa.ReduceOp.add)
                thr_block("o", sums_o[:, :])

                # ---- x quantization (loads on SP after the a0 weight
                # stream; scale-copy + rne on DVE so the Act queue stays
                # free for the Sign/transpose/flush chain) ----
                with tc.tile_pool(name="xst", bufs=3) as xst:
                    for m in range(MT):
                        xt = xst.tile([128, D], F32, tag="x_in")
                        nc.sync.dma_start(out=xt[:, :], in_=Xv[m])
                        gx = gx_l[m]
                        nc.vector.tensor_reduce(out=gx[:, :], in_=xt[:, :],
                                                axis=AX.X, op=OP.max,
                                                apply_absolute_value=True)
                        nc.vector.tensor_scalar_max(out=gx[:, :],
                                                    in0=gx[:, :],
                                                    scalar1=1e-5)
                        rcp = xst.tile([128, 1], F32, tag="rcpx")
                        nc.vector.reciprocal(out=rcp[:, :], in_=gx[:, :])
                        sx = xst.tile([128, 1], F32, tag="sx")
                        nc.vector.tensor_scalar_mul(out=sx[:, :],
                                                    in0=rcp[:, :],
                                                    scalar1=127.0)
                        # k_x = rne(x * sx) -> bf16 (exact small ints)
                        xs = xst.tile([128, D], F32, tag="x_sc")
                        nc.vector.tensor_scalar_mul(out=xs[:, :],
                                                    in0=xt[:, :],
                                                    scalar1=sx[:, :])
                        kx = xst.tile([128, D], BF16, tag="kx")
                        nc.vector.tensor_scalar(out=kx[:, :], in0=xs[:, :],
                                                scalar1=MAGIC, scalar2=MAGIC,
                                                op0=OP.add, op1=OP.subtract)
                        nc.sync.dma_start(out=kxT[:, :,
                                                  m * 128:(m + 1) * 128],
                                          in_=kx[:, :], transpose=True)

                # per-token eviction scales; /254 folds the ternary 2x
                for m in range(MT):
                    nc.vector.tensor_scalar(out=s1[m][:, :],
                                            in0=gx_l[m][:, :],
                                            scalar1=gam["g"][:, :],
                                            scalar2=1.0 / 254.0,
                                            op0=OP.mult, op1=OP.mult)
                    s2 = wp.tile([128, 1], F32, tag="s2tmp")
                    nc.vector.tensor_scalar(out=s2[:, :], in0=gx_l[m][:, :],
                                            scalar1=gam["v"][:, :],
                                            scalar2=1.0 / 254.0,
                                            op0=OP.mult, op1=OP.mult)
                    nc.vector.tensor_mul(out=s12[m][:, :], in0=s1[m][:, :],
                                         in1=s2[:, :])

                # second gate/val halves -> combined AllGather
                tern_unit(Gv, R2, RSH, D, QD, thr["g"], nthr["g"], False,
                          gvshv[1][0])
                tern_unit(Vv, R2, RSH, D, QD, thr["v"], nthr["v"], False,
                          gvshv[1][1])
                nc.gpsimd.collective_compute(
                    "AllGather", OP.bypass, replica_groups=RG,
                    ins=[gvsh[1].opt()], outs=[gvall[1].opt()])

                # out_w shard: [D, SH] fp32 -> ternary [SH, D] (h-major),
                # d-halves combined into one AllGather
                for a in range(2):
                    tern_unit(Owv, a * (RD // 2), (a + 1) * (RD // 2), SH,
                              HB, thr["o"], nthr["o"], False, oshv[a])
                nc.gpsimd.collective_compute(
                    "AllGather", OP.bypass, replica_groups=RG,
                    ins=[osh.opt()], outs=[oall.opt()])

            # ---------------- mm1: gate/val matmuls + h ----------------
            # panel (s, a) covers h rows [s*SH + a*512, ...+512)
            with tc.tile_pool(name="m1p", bufs=2,
                              side="right") as m1p:
                gvallv = [gvall[a].rearrange("s t (q p) c -> (s t) p q c",
                                             p=128) for a in range(2)]
                for a in range(2):
                    for s in range(NS):
                        n = 2 * s + a  # h 512-panel index
                        wg_s = m1p.tile([128, KD, 512], FP8, tag="wg_s")
                        wv_s = m1p.tile([128, KD, 512], FP8, tag="wv_s")
                        nc.sync.dma_start(out=wg_s[:, :, :],
                                          in_=gvallv[a][2 * s])
                        nc.sync.dma_start(out=wv_s[:, :, :],
                                          in_=gvallv[a][2 * s + 1])
                        for half in range(MT // MHALF):
                            ms = range(half * MHALF, (half + 1) * MHALF)
                            pg = {m: psp.tile([128, 512], F32, tag="ps",
                                              name=f"pg{n}_{m}")
                                  for m in ms}
                            pv = {m: psp.tile([128, 512], F32, tag="ps",
                                              name=f"pv{n}_{m}")
                                  for m in ms}
                            for k in range(KD):
                                for m in ms:
                                    lhsT = kxT[:, k, m * 128:(m + 1) * 128]
                                    nc.tensor.matmul(pg[m][:, :], lhsT=lhsT,
                                                     rhs=wg_s[:, k, :],
                                                     start=(k == 0),
                                                     stop=(k == KD - 1))
                                    nc.tensor.matmul(pv[m][:, :], lhsT=lhsT,
                                                     rhs=wv_s[:, k, :],
                                                     start=(k == 0),
                                                     stop=(k == KD - 1))
                            for m in ms:
                                A = m1p.tile([128, 512], F32, tag="Asb",
                                             bufs=MHALF + 2, name=f"A{n}_{m}")
                                nc.scalar.activation(out=A[:, :],
                                                     in_=pg[m][:, :],
                                                     func=AF.Sigmoid,
                                                     scale=s1[m][:, :])
                                B = m1p.tile([128, 512], F32, tag="Bsb",
                                             bufs=MHALF + 2, name=f"B{n}_{m}")
                                nc.scalar.activation(out=B[:, :],
                                                     in_=pg[m][:, :],
                                                     func=AF.Copy,
                                                     scale=s12[m][:, :])
                                tmp = m1p.tile([128, 512], F32, tag="tmp",
                                               bufs=4, name=f"tmp{n}_{m}")
                                nc.vector.tensor_mul(out=tmp[:, :],
                                                     in0=pv[m][:, :],
                                                     in1=B[:, :])
                                hs = m1p.tile([128, 512], F32, tag="hsl",
                                              bufs=4, name=f"hs{n}_{m}")
                                nc.vector.tensor_mul(out=hs[:, :],
                                                     in0=A[:, :],
                                                     in1=tmp[:, :])
                                nc.vector.tensor_reduce(
                                    out=hp[m][:, n:n + 1], in_=hs[:, :],
                                    axis=AX.X, op=OP.max,
                                    apply_absolute_value=True)
                                nc.sync.dma_start(
                                    out=h_d[m, :, n * 512:(n + 1) * 512],
                                    in_=hs[:, :])

        # ---------------- h quantization + mm2 (interleaved per token
        # chunk: h-quant(m+1) overlaps mm2(m) on the PE) ----------------
        with tc.tile_pool(name="khp", bufs=1,
                          side="left") as khp:
            # mm2 weights resident in SBUF (16 MiB fp8 = 128KiB/partition),
            # loaded in 8 k-chunks so mm2(m=0) can start on chunk 0.
            # oall as [8192 h, 2048 d]: row h = s*SH + hb*128 + p
            wo_all = khp.tile([128, KH, D], FP8, tag="wo_all")
            # one k-chunk of 8 == one shard s: index per (shard, d-half)
            oallv = oall.rearrange("s t (k p) c -> s t p k c", p=128)
            D2 = D // 2
            for kk in range(8):
                sl = slice(kk * (KH // 8), (kk + 1) * (KH // 8))
                for a in range(2):
                    nc.sync.dma_start(
                        out=wo_all[:, sl, a * D2:(a + 1) * D2],
                        in_=oallv[kk][a])
            with tc.tile_pool(name="hqp", bufs=3,
                              side="left") as hqp:
                for m in range(MT):
                    nc.vector.tensor_reduce(out=hmax[m][:, :],
                                            in_=hp[m][:, :], axis=AX.X,
                                            op=OP.max)
                    gh = hqp.tile([128, 1], F32, tag="gh")
                    nc.vector.tensor_scalar_max(out=gh[:, :],
                                                in0=hmax[m][:, :],
                                                scalar1=1e-5)
                    rch = hqp.tile([128, 1], F32, tag="rch")
                    nc.vector.reciprocal(out=rch[:, :], in_=gh[:, :])
                    sh = hqp.tile([128, 1], F32, tag="sh")
                    nc.vector.tensor_scalar_mul(out=sh[:, :], in0=rch[:, :],
                                                scalar1=127.0)
                    nc.vector.tensor_scalar(out=s_out[m][:, :],
                                            in0=gh[:, :],
                                            scalar1=gam["o"][:, :],
                                            scalar2=1.0 / 254.0,
                                            op0=OP.mult, op1=OP.mult)
                    kT = hqp.tile([128, KH, 128], BF16, tag="khT", bufs=3,
                                  name=f"khT{m}")
                    for q in range(NQ):
                        hc = hqp.tile([128, CQ], F32, tag="h_rd")
                        nc.gpsimd.dma_start(
                            out=hc[:, :],
                            in_=h_d[m, :, q * CQ:(q + 1) * CQ])
                        hsc = hqp.tile([128, CQ], F32, tag="h_sc")
                        nc.scalar.activation(out=hsc[:, :], in_=hc[:, :],
                                             func=AF.Copy, scale=sh[:, :])
                        kh = hqp.tile([128, CQ], BF16, tag="kh")
                        nc.vector.tensor_scalar(out=kh[:, :], in0=hsc[:, :],
                                                scalar1=MAGIC, scalar2=MAGIC,
                                                op0=OP.add, op1=OP.subtract)
                        nc.sync.dma_start(
                            out=kT[:, q * (CQ // 128):(q + 1) * (CQ // 128),
                                   :],
                            in_=kh[:, :], transpose=True)
                    po = [psp.tile([128, 512], F32, tag="ps",
                                   name=f"po{m}_{c}") for c in range(ND)]
                    for k in range(KH):
                        for c in range(ND):
                            nc.tensor.matmul(
                                po[c][:, :], lhsT=kT[:, k, :],
                                rhs=wo_all[:, k, c * 512:(c + 1) * 512],
                                start=(k == 0), stop=(k == KH - 1))
                    for c in range(ND):
                        ot = hqp.tile([128, 512], F32, tag="ot", bufs=4,
                                      name=f"ot{m}_{c}")
                        nc.vector.tensor_scalar_mul(out=ot[:, :],
                                                    in0=po[c][:, :],
                                                    scalar1=s_out[m][:, :])
                        nc.sync.dma_start(
                            out=Ov[m][:, c * 512:(c + 1) * 512],
                            in_=ot[:, :])


_NC_CACHE = {}


def _get_nc(T, D, H):
    key = (T, D, H)
    if key not in _NC_CACHE:
        _NC_CACHE[key] = _build(T, D, H)
    return _NC_CACHE[key]


def kernel(x, gate_w, gate_b, val_w, val_b, out_w, out_b, _trace=False):
    x = np.ascontiguousarray(np.asarray(x), dtype=np.float32)
    gate_w = np.ascontiguousarray(np.asarray(gate_w), dtype=np.float32)
    val_w = np.ascontiguousarray(np.asarray(val_w), dtype=np.float32)
    out_w = np.ascontiguousarray(np.asarray(out_w), dtype=np.float32)
    gate_b = np.asarray(gate_b)
    val_b = np.asarray(val_b)
    out_b = np.asarray(out_b)
    assert not np.any(gate_b) and not np.any(val_b), (
        "device kernel folds silu(y+b) with b=0; nonzero gate/val bias "
        "not supported")

    orig_shape = x.shape
    xf = x.reshape(-1, x.shape[-1])
    n_tok, d = xf.shape
    h = gate_w.shape[0]
    t_core = n_tok // N_CORES
    sh = h // N_CORES

    nc = _get_nc(t_core, d, h)
    in_maps = [
        {
            "x": xf[i * t_core:(i + 1) * t_core],
            "gw": gate_w[i * sh:(i + 1) * sh],
            "vw": val_w[i * sh:(i + 1) * sh],
            "ow": np.ascontiguousarray(out_w[:, i * sh:(i + 1) * sh]),
        }
        for i in range(N_CORES)
    ]
    res = run_bass_kernel_spmd(nc, in_maps, core_ids=list(range(N_CORES)),
                               trace=_trace)
    out = np.concatenate([res.results[i]["out"] for i in range(N_CORES)],
                         axis=0)
    out = out + out_b[None, :].astype(np.float32)
    kernel._last_results = res
    return out.reshape(orig_shape)


# revision 39
# speedup vs baseline: 1.2650x; 1.0382x over previous
"""BitSwiGLU Trainium2 kernel (8 NeuronCores).

Math (per bit_linear, forward values):
    gamma_x = clip(max|x_row|, 1e-5);  k = rne(x * 127/gamma_x)  in [-127,127]
    gamma_w = clip(mean|w|, 1e-5);    t = sign(w) * (|w| > 0.5*gamma_w)  in {-1,0,1}
    y = (k @ t.T) * (gamma_x*gamma_w/127) + b

k is exactly representable in bf16 and 2*t in fp8e4; the TensorEngine
accumulates (bf16 x fp8) products in fp32 PSUM, so k @ (2t).T is EXACT
integer math at full PE speed. All scales are applied per-token
(per-partition) at PSUM eviction; the ternary 2x folds into them (/254).

Sharding: data-parallel matmuls (8192 tokens split 1024/core, each core
computes its tokens against the full weights), but tensor-parallel weight
PREP: core i reads only rows [1024*i, 1024*(i+1)) of gate_w/val_w and
columns of out_w (fp32), computes |w|-sum partials (AllReduce -> exact
global gamma = mean|w|), ternarizes its shard, transposes it into matmul
layout ([d, h] for gate/val, [h, d] for out_w) via the DMA XBAR in bf16,
casts to fp8e4 and AllGathers the compact ternary shards. This removes
the 8x-redundant fp32 weight reads (2 x 192 MiB/core in the pure-DP
version) that made weight prep DMA-bound. The gate/val gathers are
split in h-halves so mm1 can start before the full gather lands.

silu(y) is computed as y * sigmoid(y) (Sigmoid on ScalarE). Biases are
zero in this problem; gate/val biases are asserted zero host-side and
out_b is added on host.
"""

import numpy as np

import concourse.bass as bass
import concourse.mybir as mybir
import concourse.tile as tile
from concourse import bacc
from concourse import bass_isa
from concourse.bass_utils import run_bass_kernel_spmd

F32 = mybir.dt.float32
BF16 = mybir.dt.bfloat16
FP8 = mybir.dt.float8e4
AF = mybir.ActivationFunctionType
OP = mybir.AluOpType
AX = mybir.AxisListType

MAGIC = 12582912.0  # 1.5 * 2**23 : (v + MAGIC) - MAGIC == rne(v) for |v| < 2**22

N_CORES = 8
RG = [[i for i in range(N_CORES)]]  # replica group: all cores


def _build(T, D, H, n_cores=N_CORES):
    """Build + compile the per-core Bass program (SPMD: all cores run the
    same program; per-core inputs differ: x token-shard + weight shards)."""
    SH = H // n_cores
    nc = bacc.Bacc("TRN2", target_bir_lowering=False, debug=False,
                   num_devices=n_cores)
    x_d = nc.dram_tensor("x", [T, D], F32, kind="ExternalInput")
    gw_d = nc.dram_tensor("gw", [SH, D], F32, kind="ExternalInput")
    vw_d = nc.dram_tensor("vw", [SH, D], F32, kind="ExternalInput")
    ow_d = nc.dram_tensor("ow", [D, SH], F32, kind="ExternalInput")
    out_d = nc.dram_tensor("out", [T, D], F32, kind="ExternalOutput")

    with tile.TileContext(nc) as tc:
        _body(tc, x_d, gw_d, vw_d, ow_d, out_d, T=T, D=D, H=H, SH=SH)
    nc.compile()
    return nc


def _body(tc, x_d, gw_d, vw_d, ow_d, out_d, *, T, D, H, SH):
    nc = tc.nc
    KD = D // 128      # contraction chunks, mm1 (16)
    KH = H // 128      # contraction chunks, mm2 (64)
    ND = D // 512      # d_out 512-chunks (mm2 output tiles) (4)
    MT = T // 128      # token chunks (8)
    RSH = SH // 128    # gate/val shard row-chunks (8)
    RD = D // 128      # out_w shard row-chunks (16)
    QD = D // 128      # d-chunks of transposed gate/val layout (16)
    HB = SH // 128     # h-blocks of transposed out_w layout (8)
    NS = N_CORES       # weight shards
    MHALF = 2          # token chunks per PSUM wave
    CQ = 512           # h-quant processing chunk
    NQ = H // CQ
    WCNT = float(H * D)  # element count of each weight matrix (mean divisor)

    Xv = x_d.ap().rearrange("(m p) d -> m p d", p=128)
    Ov = out_d.ap().rearrange("(m p) d -> m p d", p=128)
    Gv = gw_d.ap().rearrange("(r p) c -> r p c", p=128)
    Vv = vw_d.ap().rearrange("(r p) c -> r p c", p=128)
    Owv = ow_d.ap().rearrange("(r p) c -> r p c", p=128)

    with (
        tc.tile_pool(name="persist", bufs=1) as pp,
        tc.tile_pool(name="psp", bufs=8, space="PSUM") as psp,
        tc.tile_pool(name="drp", bufs=1, space="DRAM") as drp,
    ):
        # ---- DRAM scratch ----
        # own ternary shards (transposed to matmul layout, fp8 {-2,0,2});
        # gate+val share one tensor per h-half so ONE AllGather moves both
        # (the ~100us fixed ring overhead per collective dominates).
        gvsh = [drp.tile([2, D, SH // 2], FP8, tag=f"gvsh{a}",
                         name=f"gvsh{a}") for a in range(2)]
        osh = drp.tile([2, SH, D // 2], FP8, tag="osh", name="osh")
        # all-gathered ternary weights
        gvall = [drp.tile([NS, 2, D, SH // 2], FP8, tag=f"gvall{a}",
                          name=f"gvall{a}", addr_space="Shared")
                 for a in range(2)]
        oall = drp.tile([NS, 2, SH, D // 2], FP8, tag="oall", name="oall",
                        addr_space="Shared")
        # gamma partial-sum bounce buffers (gate+val in one early
        # AllReduce; out_w in a second, off the mm1 critical path)
        gvsum_in = drp.tile([128, 2], F32, tag="gvsum_in", name="gvsum_in")
        gvsum_out = drp.tile([128, 2], F32, tag="gvsum_out",
                             name="gvsum_out", addr_space="Shared")
        osum_in = drp.tile([128, 1], F32, tag="osum_in", name="osum_in")
        osum_out = drp.tile([128, 1], F32, tag="osum_out", name="osum_out",
                            addr_space="Shared")
        # h spill
        h_d = drp.tile([MT, 128, H], F32, tag="h", name="h_d")

        # ---- persistent SBUF ----
        s1, s12, gx_l, hmax, s_out = [], [], [], [], []
        for m in range(MT):
            for nm, lst in (("s1", s1), ("s12", s12), ("gx", gx_l),
                            ("hmax", hmax), ("so", s_out)):
                t = pp.tile([128, 1], F32, tag=f"{nm}{m}", name=f"{nm}{m}")
                lst.append(t)
        hp = [pp.tile([128, H // 512], F32, tag=f"hp{m}", name=f"hp{m}")
              for m in range(MT)]
        # per-partition |w| partial sums: cols 0..RSH-1 gate, RSH..2RSH-1
        # val, 2RSH..2RSH+RD-1 out
        parts = pp.tile([128, 2 * RSH + RD], F32, tag="parts", name="parts")
        sums_gv = pp.tile([128, 2], F32, tag="sums_gv", name="sums_gv")
        sums_o = pp.tile([128, 1], F32, tag="sums_o", name="sums_o")
        gam, thr, nthr = {}, {}, {}
        for j, nm in enumerate(("g", "v", "o")):
            gam[nm] = pp.tile([128, 1], F32, tag=f"gam_{nm}", name=f"gam_{nm}")
            thr[nm] = pp.tile([128, 1], F32, tag=f"thr_{nm}", name=f"thr_{nm}")
            nthr[nm] = pp.tile([128, 1], F32, tag=f"nthr_{nm}",
                               name=f"nthr_{nm}")

        with tc.tile_pool(name="kxp", bufs=1) as kxp:
            # kxT[p=d, k, t] = k_x[t, k*128+p]
            kxT = kxp.tile([128, KD, T], BF16, tag="kxT")

            with tc.tile_pool(name="wp", bufs=3) as wp:
                # ---- |w| partial sums of the own shard ----
                def gamma_cols(Wv, R, C, col0):
                    for r in range(R):
                        wt = wp.tile([128, 2048], F32, tag="w_in")
                        nc.sync.dma_start(out=wt[:, :C], in_=Wv[r])
                        nc.vector.tensor_reduce(
                            out=parts[:, col0 + r:col0 + r + 1],
                            in_=wt[:, :C], axis=AX.X, op=OP.add,
                            apply_absolute_value=True)

                def thr_block(nm, src_ap):
                    nc.vector.tensor_scalar(out=gam[nm][:, :], in0=src_ap,
                                            scalar1=1.0 / WCNT,
                                            scalar2=1e-5, op0=OP.mult,
                                            op1=OP.max)
                    nc.vector.tensor_scalar_mul(out=thr[nm][:, :],
                                                in0=gam[nm][:, :],
                                                scalar1=0.5)
                    nc.vector.tensor_scalar_mul(out=nthr[nm][:, :],
                                                in0=thr[nm][:, :],
                                                scalar1=-1.0)

                # gate+val gamma partials -> one small AllReduce; all the
                # scalar-sized DMAs ride the idle gpsimd queue so they
                # never head-of-line block the SP weight stream
                gamma_cols(Gv, RSH, D, 0)
                gamma_cols(Vv, RSH, D, RSH)
                psum_gv = pp.tile([128, 2], F32, tag="psum_gv",
                                  name="psum_gv")
                nc.vector.tensor_reduce(out=psum_gv[:, 0:1],
                                        in_=parts[:, 0:RSH], axis=AX.X,
                                        op=OP.add)
                nc.vector.tensor_reduce(out=psum_gv[:, 1:2],
                                        in_=parts[:, RSH:2 * RSH],
                                        axis=AX.X, op=OP.add)
                nc.gpsimd.dma_start(out=gvsum_in[:, :], in_=psum_gv[:, :])
                nc.gpsimd.collective_compute(
                    "AllReduce", OP.add, replica_groups=RG,
                    ins=[gvsum_in.opt()], outs=[gvsum_out.opt()])
                nc.gpsimd.dma_start(out=sums_gv[:, :], in_=gvsum_out[:, :])
                nc.gpsimd.partition_all_reduce(sums_gv[:, :],
                                               sums_gv[:, :], 128,
                                               bass_isa.ReduceOp.add)
                thr_block("g", sums_gv[:, 0:1])
                thr_block("v", sums_gv[:, 1:2])

                # ternarize own shard rows -> bf16, XBAR-transpose into a
                # per-half staging tile, cast to fp8, write to DRAM shard
                def tern_rows(Wv, r0, r1, C, wT, th, nth, dve):
                    for r in range(r0, r1):
                        wt = wp.tile([128, 2048], F32, tag="w_in")
                        nc.sync.dma_start(out=wt[:, :C], in_=Wv[r])
                        tq = wp.tile([128, 2048], BF16, tag="q_tq")
                        if dve:
                            mp = wp.tile([128, 2048], BF16, tag="q_mm",
                                         bufs=2)
                            nc.vector.tensor_scalar(
                                out=mp[:, :C], in0=wt[:, :C],
                                scalar1=th[:, :], scalar2=2.0,
                                op0=OP.is_gt, op1=OP.mult)
                            mn = wp.tile([128, 2048], BF16, tag="q_mm",
                                         bufs=2)
                            nc.vector.tensor_scalar(
                                out=mn[:, :C], in0=wt[:, :C],
                                scalar1=nth[:, :], scalar2=2.0,
                                op0=OP.is_lt, op1=OP.mult)
                            nc.vector.tensor_sub(out=tq[:, :C],
                                                 in0=mp[:, :C],
                                                 in1=mn[:, :C])
                        else:
                            sp = wp.tile([128, 2048], BF16, tag="q_sp",
                                         bufs=2)
                            nc.scalar.activation(out=sp[:, :C],
                                                 in_=wt[:, :C],
                                                 func=AF.Sign,
                                                 bias=nth[:, :])
                            sn = wp.tile([128, 2048], BF16, tag="q_sn",
                                         bufs=2)
                            nc.scalar.activation(out=sn[:, :C],
                                                 in_=wt[:, :C],
                                                 func=AF.Sign,
                                                 bias=th[:, :])
                            nc.vector.tensor_add(out=tq[:, :C],
                                                 in0=sp[:, :C],
                                                 in1=sn[:, :C])
                        # multi-tile XBAR transpose: out[p, q, c] =
                        # tq[c, q*128+p]
                        cs = (r - r0) * 128
                        nc.scalar.dma_start(out=wT[:, :, cs:cs + 128],
                                            in_=tq[:, :C], transpose=True)

                def tern_unit(Wv, r0, r1, C, nq, th, nth, dve, dst_view):
                    ncols = (r1 - r0) * 128
                    wT = wp.tile([128, nq * ncols], BF16, tag="wT", bufs=2)
                    wTv = wT.rearrange("p (q c) -> p q c", c=ncols)
                    tern_rows(Wv, r0, r1, C, wTv, th, nth, dve)
                    f8 = wp.tile([128, nq * ncols], FP8, tag="f8w", bufs=2)
                    f8v = f8.rearrange("p (q c) -> p q c", c=ncols)
                    nc.scalar.activation(out=f8v[:, :, :], in_=wTv[:, :, :],
                                         func=AF.Copy)
                    nc.scalar.dma_start(out=dst_view, in_=f8v[:, :, :])

                R2 = RSH // 2
                gvshv = [gvsh[a].rearrange("t (q p) c -> t p q c", p=128)
                         for a in range(2)]
                oshv = osh.rearrange("t (q p) c -> t p q c", p=128)

                # first gate/val halves -> ONE combined AllGather (the mm1
                # critical path)
                tern_unit(Gv, 0, R2, D, QD, thr["g"], nthr["g"], False,
                          gvshv[0][0])
                tern_unit(Vv, 0, R2, D, QD, thr["v"], nthr["v"], False,
                          gvshv[0][1])
                nc.gpsimd.collective_compute(
                    "AllGather", OP.bypass, replica_groups=RG,
                    ins=[gvsh[0].opt()], outs=[gvall[0].opt()])

                # out_w gamma + its own AllReduce (cc-pipe slot after the
                # first gather)
                gamma_cols(Owv, RD, SH, 2 * RSH)
                nc.vector.tensor_reduce(out=sums_o[:, :],
                                        in_=parts[:, 2 * RSH:2 * RSH + RD],
                                        axis=AX.X, op=OP.add)
                nc.gpsimd.dma_start(out=osum_in[:, :], in_=sums_o[:, :])
                nc.gpsimd.collective_compute(
                    "AllReduce", OP.add, replica_groups=RG,
                    ins=[osum_in.opt()], outs=[osum_out.opt()])
                nc.gpsimd.dma_start(out=sums_o[:, :], in_=osum_out[:, :])
                nc.gpsimd.partition_all_reduce(sums_o[:, :], sums_o[:, :],
                                               128, bass_isa.ReduceOp.add)
                thr_block("o", sums_o[:, :])

                # ---- x quantization (loads on SP after the a0 weight
                # stream; scale-copy + rne on DVE so the Act queue stays
                # free for the Sign/transpose/flush chain) ----
                with tc.tile_pool(name="xst", bufs=3) as xst:
                    for m in range(MT):
                        xt = xst.tile([128, D], F32, tag="x_in")
                        nc.sync.dma_start(out=xt[:, :], in_=Xv[m])
                        gx = gx_l[m]
                        nc.vector.tensor_reduce(out=gx[:, :], in_=xt[:, :],
                                                axis=AX.X, op=OP.max,
                                                apply_absolute_value=True)
                        nc.vector.tensor_scalar_max(out=gx[:, :],
                                                    in0=gx[:, :],
                                                    scalar1=1e-5)
                        rcp = xst.tile([128, 1], F32, tag="rcpx")
                        nc.vector.reciprocal(out=rcp[:, :], in_=gx[:, :])
                        sx = xst.tile([128, 1], F32, tag="sx")
                        nc.vector.tensor_scalar_mul(out=sx[:, :],
                                                    in0=rcp[:, :],
                                                    scalar1=127.0)
                        # k_x = rne(x * sx) -> bf16 (exact small ints)
                        xs = xst.tile([128, D], F32, tag="x_sc")
                        nc.vector.tensor_scalar_mul(out=xs[:, :],
                                                    in0=xt[:, :],
                                                    scalar1=sx[:, :])
                        kx = xst.tile([128, D], BF16, tag="kx")
                        nc.vector.tensor_scalar(out=kx[:, :], in0=xs[:, :],
                                                scalar1=MAGIC, scalar2=MAGIC,
                                                op0=OP.add, op1=OP.subtract)
                        nc.sync.dma_start(out=kxT[:, :,
                                                  m * 128:(m + 1) * 128],
                                          in_=kx[:, :], transpose=True)

                # per-token eviction scales; /254 folds the ternary 2x
                for m in range(MT):
                    nc.vector.tensor_scalar(out=s1[m][:, :],
                                            in0=gx_l[m][:, :],
                                            scalar1=gam["g"][:, :],
                                            scalar2=1.0 / 254.0,
                                            op0=OP.mult, op1=OP.mult)
                    s2 = wp.tile([128, 1], F32, tag="s2tmp")
                    nc.vector.tensor_scalar(out=s2[:, :], in0=gx_l[m][:, :],
                                            scalar1=gam["v"][:, :],
                                            scalar2=1.0 / 254.0,
                                            op0=OP.mult, op1=OP.mult)
                    nc.vector.tensor_mul(out=s12[m][:, :], in0=s1[m][:, :],
                                         in1=s2[:, :])

                # second gate/val halves -> combined AllGather
                tern_unit(Gv, R2, RSH, D, QD, thr["g"], nthr["g"], False,
                          gvshv[1][0])
                tern_unit(Vv, R2, RSH, D, QD, thr["v"], nthr["v"], False,
                          gvshv[1][1])
                nc.gpsimd.collective_compute(
                    "AllGather", OP.bypass, replica_groups=RG,
                    ins=[gvsh[1].opt()], outs=[gvall[1].opt()])

                # out_w shard: [D, SH] fp32 -> ternary [SH, D] (h-major),
                # d-halves combined into one AllGather
                for a in range(2):
                    tern_unit(Owv, a * (RD // 2), (a + 1) * (RD // 2), SH,
                              HB, thr["o"], nthr["o"], False, oshv[a])
                nc.gpsimd.collective_compute(
                    "AllGather", OP.bypass, replica_groups=RG,
                    ins=[osh.opt()], outs=[oall.opt()])

            # ---------------- mm1: gate/val matmuls + h ----------------
            # panel (s, a) covers h rows [s*SH + a*512, ...+512)
            with tc.tile_pool(name="m1p", bufs=2) as m1p:
                gvallv = [gvall[a].rearrange("s t (q p) c -> (s t) p q c",
                                             p=128) for a in range(2)]
                for a in range(2):
                    for s in range(NS):
                        n = 2 * s + a  # h 512-panel index
                        wg_s = m1p.tile([128, KD, 512], FP8, tag="wg_s")
                        wv_s = m1p.tile([128, KD, 512], FP8, tag="wv_s")
                        nc.sync.dma_start(out=wg_s[:, :, :],
                                          in_=gvallv[a][2 * s])
                        nc.sync.dma_start(out=wv_s[:, :, :],
                                          in_=gvallv[a][2 * s + 1])
                        for half in range(MT // MHALF):
                            ms = range(half * MHALF, (half + 1) * MHALF)
                            pg = {m: psp.tile([128, 512], F32, tag="ps",
                                              name=f"pg{n}_{m}")
                                  for m in ms}
                            pv = {m: psp.tile([128, 512], F32, tag="ps",
                                              name=f"pv{n}_{m}")
                                  for m in ms}
                            for k in range(KD):
                                for m in ms:
                                    lhsT = kxT[:, k, m * 128:(m + 1) * 128]
                                    nc.tensor.matmul(pg[m][:, :], lhsT=lhsT,
                                                     rhs=wg_s[:, k, :],
                                                     start=(k == 0),
                                                     stop=(k == KD - 1))
                                    nc.tensor.matmul(pv[m][:, :], lhsT=lhsT,
                                                     rhs=wv_s[:, k, :],
                                                     start=(k == 0),
                                                     stop=(k == KD - 1))
                            for m in ms:
                                A = m1p.tile([128, 512], F32, tag="Asb",
                                             bufs=MHALF + 2, name=f"A{n}_{m}")
                                nc.scalar.activation(out=A[:, :],
                                                     in_=pg[m][:, :],
                                                     func=AF.Sigmoid,
                                                     scale=s1[m][:, :])
                                B = m1p.tile([128, 512], F32, tag="Bsb",
                                             bufs=MHALF + 2, name=f"B{n}_{m}")
                                nc.scalar.activation(out=B[:, :],
                                                     in_=pg[m][:, :],
                                                     func=AF.Copy,
                                                     scale=s12[m][:, :])
                                tmp = m1p.tile([128, 512], F32, tag="tmp",
                                               bufs=4, name=f"tmp{n}_{m}")
                                nc.vector.tensor_mul(out=tmp[:, :],
                                                     in0=pv[m][:, :],
                                                     in1=B[:, :])
                                hs = m1p.tile([128, 512], F32, tag="hsl",
                                              bufs=4, name=f"hs{n}_{m}")
                                nc.vector.tensor_mul(out=hs[:, :],
                                                     in0=A[:, :],
                                                     in1=tmp[:, :])
                                nc.vector.tensor_reduce(
                                    out=hp[m][:, n:n + 1], in_=hs[:, :],
                                    axis=AX.X, op=OP.max,
                                    apply_absolute_value=True)
                                nc.sync.dma_start(
                                    out=h_d[m, :, n * 512:(n + 1) * 512],
                                    in_=hs[:, :])

        # ---------------- h quantization + mm2 (interleaved per token
        # chunk: h-quant(m+1) overlaps mm2(m) on the PE) ----------------
        with tc.tile_pool(name="khp", bufs=1) as khp:
            # mm2 weights resident in SBUF (16 MiB fp8 = 128KiB/partition),
            # loaded in 8 k-chunks so mm2(m=0) can start on chunk 0.
            # oall as [8192 h, 2048 d]: row h = s*SH + hb*128 + p
            wo_all = khp.tile([128, KH, D], FP8, tag="wo_all")
            # one k-chunk of 8 == one shard s: index per (shard, d-half)
            oallv = oall.rearrange("s t (k p) c -> s t p k c", p=128)
            D2 = D // 2
            for kk in range(8):
                sl = slice(kk * (KH // 8), (kk + 1) * (KH // 8))
                for a in range(2):
                    nc.sync.dma_start(
                        out=wo_all[:, sl, a * D2:(a + 1) * D2],
                        in_=oallv[kk][a])
            with tc.tile_pool(name="hqp", bufs=3) as hqp:
                for m in range(MT):
                    nc.vector.tensor_reduce(out=hmax[m][:, :],
                                            in_=hp[m][:, :], axis=AX.X,
                                            op=OP.max)
                    gh = hqp.tile([128, 1], F32, tag="gh")
                    nc.vector.tensor_scalar_max(out=gh[:, :],
                                                in0=hmax[m][:, :],
                                                scalar1=1e-5)
                    rch = hqp.tile([128, 1], F32, tag="rch")
                    nc.vector.reciprocal(out=rch[:, :], in_=gh[:, :])
                    sh = hqp.tile([128, 1], F32, tag="sh")
                    nc.vector.tensor_scalar_mul(out=sh[:, :], in0=rch[:, :],
                                                scalar1=127.0)
                    nc.vector.tensor_scalar(out=s_out[m][:, :],
                                            in0=gh[:, :],
                                            scalar1=gam["o"][:, :],
                                            scalar2=1.0 / 254.0,
                                            op0=OP.mult, op1=OP.mult)
                    kT = hqp.tile([128, KH, 128], BF16, tag="khT", bufs=3,
                                  name=f"khT{m}")
                    for q in range(NQ):
                        hc = hqp.tile([128, CQ], F32, tag="h_rd")
                        nc.gpsimd.dma_start(
                            out=hc[:, :],
                            in_=h_d[m, :, q * CQ:(q + 1) * CQ])
                        hsc = hqp.tile([128, CQ], F32, tag="h_sc")
                        nc.scalar.activation(out=hsc[:, :], in_=hc[:, :],
                                             func=AF.Copy, scale=sh[:, :])
                        kh = hqp.tile([128, CQ], BF16, tag="kh")
                        nc.vector.tensor_scalar(out=kh[:, :], in0=hsc[:, :],
                                                scalar1=MAGIC, scalar2=MAGIC,
                                                op0=OP.add, op1=OP.subtract)
                        nc.sync.dma_start(
                            out=kT[:, q * (CQ // 128):(q + 1) * (CQ // 128),
                                   :],
                            in_=kh[:, :], transpose=True)
                    po = [psp.tile([128, 512], F32, tag="ps",
                                   name=f"po{m}_{c}") for c in range(ND)]
                    for k in range(KH):
                        for c in range(ND):
                            nc.tensor.matmul(
                                po[c][:, :], lhsT=kT[:, k, :],
                                rhs=wo_all[:, k, c * 512:(c + 1) * 512],
                                start=(k == 0), stop=(k == KH - 1))
                    for c in range(ND):
                        ot = hqp.tile([128, 512], F32, tag="ot", bufs=4,
                                      name=f"ot{m}_{c}")
                        nc.vector.tensor_scalar_mul(out=ot[:, :],
                                                    in0=po[c][:, :],
                                                    scalar1=s_out[m][:, :])
                        nc.sync.dma_start(
                            out=Ov[m][:, c * 512:(c + 1) * 512],
                            in_=ot[:, :])


_NC_CACHE = {}


def _get_nc(T, D, H):
    key = (T, D, H)
    if key not in _NC_CACHE:
        _NC_CACHE[key] = _build(T, D, H)
    return _NC_CACHE[key]


def kernel(x, gate_w, gate_b, val_w, val_b, out_w, out_b, _trace=False):
    x = np.ascontiguousarray(np.asarray(x), dtype=np.float32)
    gate_w = np.ascontiguousarray(np.asarray(gate_w), dtype=np.float32)
    val_w = np.ascontiguousarray(np.asarray(val_w), dtype=np.float32)
    out_w = np.ascontiguousarray(np.asarray(out_w), dtype=np.float32)
    gate_b = np.asarray(gate_b)
    val_b = np.asarray(val_b)
    out_b = np.asarray(out_b)
    assert not np.any(gate_b) and not np.any(val_b), (
        "device kernel folds silu(y+b) with b=0; nonzero gate/val bias "
        "not supported")

    orig_shape = x.shape
    xf = x.reshape(-1, x.shape[-1])
    n_tok, d = xf.shape
    h = gate_w.shape[0]
    t_core = n_tok // N_CORES
    sh = h // N_CORES

    nc = _get_nc(t_core, d, h)
    in_maps = [
        {
            "x": xf[i * t_core:(i + 1) * t_core],
            "gw": gate_w[i * sh:(i + 1) * sh],
            "vw": val_w[i * sh:(i + 1) * sh],
            "ow": np.ascontiguousarray(out_w[:, i * sh:(i + 1) * sh]),
        }
        for i in range(N_CORES)
    ]
    res = run_bass_kernel_spmd(nc, in_maps, core_ids=list(range(N_CORES)),
                               trace=_trace)
    out = np.concatenate([res.results[i]["out"] for i in range(N_CORES)],
                         axis=0)
    out = out + out_b[None, :].astype(np.float32)
    kernel._last_results = res
    return out.reshape(orig_shape)
